# revision 27
# baseline (speedup 1.0000x reference)
"""AdaptiveStateSelector (gated 3-expert SS2D / Mamba-style 2D selective scan) on 8 TRN2 cores.

Sharding: core c -> (b = c//4, unit u = c%4). Units own expert state-slices:
  u0: expert0 n=0:8 (padded to 16 lanes), u1: expert1 n=0:16, u2: expert2 n=0:16, u3: expert2 n=16:32.
Each core computes all 4 scan directions for its slice; direction flips/transposes are
handled in-core (negative-stride APs for flipped scans, a transposed copy for the
vertical scans). Partial gated outputs are combined with one ReduceScatter per
batch-group (experts as slots, masked by per-core ownership data), after which every
core runs the epilogue (LayerNorm, silu gate, gated out-projection) on its L-quarter.
"""

import numpy as np

B, HS, WS, C = 2, 48, 48, 96
L = HS * WS            # 2304
D2 = 192               # expanded channels (2*96)
NS = 16                # n-lanes per core (padded)
R = 6                  # dt rank
CR = R + 2 * NS        # 38 live rows of the sliced x-proj
CRP = 96               # padded to 32-aligned sections: dt@0, B@32, C@64
QL = L // 4            # 576, L-quarter per core in the epilogue
PARTS = [(0, 128), (128, 64)]     # d=192 split into partition tiles
FCH = [(0, 512), (512, 512), (1024, 512), (1536, 512), (2048, 256)]
FCQ = [(0, 512), (512, 64)]       # 576 in PSUM-bank chunks
UNIT_EXPERT = [0, 1, 2, 2]
UNIT_NSLICE = [(0, 8), (0, 16), (0, 16), (16, 32)]
EXPERT_N = [8, 16, 32]
EPS = 1e-5


def build_graph():
    import concourse.bass as bass
    import concourse.tile as tile
    from concourse import bacc, mybir

    f32 = mybir.dt.float32
    bf16 = mybir.dt.bfloat16
    AF = mybir.ActivationFunctionType
    OP = mybir.AluOpType

    nc = bacc.Bacc("TRN2", target_bir_lowering=False, debug=False, num_devices=8)

    def param(name, shape, dt=f32, out=False):
        return nc.declare_dram_parameter(name, list(shape), dt, isOutput=out)

    params = dict(
        xT=param("xT", [C, L]),
        xTq=param("xTq", [C, QL]),
        wxi=param("wxi", [C, D2]),
        wz=param("wz", [C, 3 * D2]),
        w9=param("w9", [128, 2, 9]),
        convb=param("convb", [128, 2, 1]),
        convbn=param("convbn", [128, 2, 1]),
        xproj=param("xproj", [128, 4, 2, CRP], bf16),
        dtw=param("dtw", [R, 4, D2], bf16),
        dtb=param("dtb", [128, 4, 2, 1]),
        Am=param("Am", [128, 4, 2, NS]),
        dsum=param("dsum", [128, 2, 1]),
        sel16=param("sel16", [NS, NS, 128], bf16),
        ones1=param("ones1", [1, 128]),
        onesP=param("onesP", [128, 1]),
        emask=param("emask", [128, 3, 1]),
        lnw=param("lnw", [128, 3, 2, 1]),
        lnb=param("lnb", [128, 3, 2, 1]),
        outw=param("outw", [128, 3, 2, C]),
        gw1=param("gw1", [C, 24]),
        gb1=param("gb1", [24, 1]),
        gw2=param("gw2", [24, 3]),
        gb2=param("gb2", [1, 3]),
        epsv=param("epsv", [128, 1]),
        out_ext=param("out", [C, QL], out=True),
    )
    params["rs_in"] = nc.dram_tensor("rs_in", [4, 3, 2, 128, QL], bf16)
    params["rs_out"] = nc.dram_tensor("rs_out", [3, 2, 128, QL], bf16)

    with tile.TileContext(nc) as tc:
        _build(nc, tc, bass, mybir, tile, f32, bf16, AF, OP, params)
    nc.compile()
    return nc


def _build(nc, tc, bass, mybir, tile, f32, bf16, AF, OP, T):
    from contextlib import ExitStack

    ctx = ExitStack()
    AF_ = AF
    P128 = [128, Ellipsis]

    consts = ctx.enter_context(tc.tile_pool(name="consts", bufs=1))
    big = ctx.enter_context(tc.tile_pool(name="big", bufs=1))
    mm_psum = ctx.enter_context(tc.tile_pool(name="mm_psum", bufs=3, space="PSUM"))

    # ---- load constants to SBUF (all padded to 128 partitions) ----
    def load_const(pname, dt=f32):
        p = T[pname]
        t = consts.tile(list(p.shape), dt, padded_shape=P128, name=f"s_{pname}")
        nc.sync.dma_start(t[:], p[:])
        return t

    s_xT = load_const("xT")
    s_xTq = load_const("xTq")
    s_wxi = load_const("wxi")
    s_wz = load_const("wz")
    s_w9 = load_const("w9")
    s_convb = load_const("convb")
    s_convbn = load_const("convbn")
    s_xproj = load_const("xproj", bf16)
    s_dtw = load_const("dtw", bf16)
    s_dtb = load_const("dtb")
    s_Am = load_const("Am")
    s_dsum = load_const("dsum")
    s_sel = load_const("sel16", bf16)
    s_ones1 = load_const("ones1")
    s_onesP = load_const("onesP")
    s_emask = load_const("emask")
    s_lnw = load_const("lnw")
    s_lnb = load_const("lnb")
    s_outw = load_const("outw")
    s_gw1 = load_const("gw1")
    s_gb1 = load_const("gb1")
    s_gw2 = load_const("gw2")
    s_gb2 = load_const("gb2")
    s_epsv = load_const("epsv")
    out_ext, rs_in, rs_out = T["out_ext"], T["rs_in"], T["rs_out"]

    # ---- gating network (redundant on every core) ----
    gp = ctx.enter_context(tc.tile_pool(name="gates", bufs=1))
    gates_ps_ctx = tc.tile_pool(name="gates_ps", bufs=1, space="PSUM")
    gp_ps = gates_ps_ctx.__enter__()
    pooled = gp.tile([C, 1], f32, padded_shape=P128)
    nc.vector.tensor_reduce(pooled[:], s_xT[:], axis=mybir.AxisListType.X, op=OP.add)
    h1p = gp_ps.tile([24, 1], f32, padded_shape=P128)
    nc.tensor.matmul(h1p[:], s_gw1[:], pooled[:], start=True, stop=True)
    h1 = gp.tile([24, 1], f32, padded_shape=P128)
    nc.scalar.activation(h1[:], h1p[:], AF_.Relu, bias=s_gb1[:], scale=1.0 / L)
    logp = gp_ps.tile([1, 3], f32, padded_shape=P128)
    nc.tensor.matmul(logp[:], h1[:], s_gw2[:], start=True, stop=True)
    logits = gp.tile([1, 3], f32, padded_shape=P128)
    nc.vector.tensor_add(logits[:], logp[:], s_gb2[:])
    lmax = gp.tile([1, 1], f32, padded_shape=P128)
    nc.vector.tensor_reduce(lmax[:], logits[:], axis=mybir.AxisListType.X, op=OP.max,
                            negate=True)
    elog = gp.tile([1, 3], f32, padded_shape=P128)
    nc.scalar.activation(elog[:], logits[:], AF_.Exp, bias=lmax[:])
    esum = gp.tile([1, 1], f32, padded_shape=P128)
    nc.vector.tensor_reduce(esum[:], elog[:], axis=mybir.AxisListType.X, op=OP.add)
    einv = gp.tile([1, 1], f32, padded_shape=P128)
    nc.vector.reciprocal(einv[:], esum[:])
    gates = gp.tile([1, 3], f32, padded_shape=P128)
    nc.vector.tensor_scalar_mul(gates[:], elog[:], einv[:])
    gbc_ps = gp_ps.tile([128, 3], f32)
    nc.tensor.matmul(gbc_ps[:], s_ones1[:], gates[:], start=True, stop=True)
    gbc = gp.tile([128, 3], f32)
    nc.scalar.copy(gbc[:], gbc_ps[:])
    gates_ps_ctx.__exit__(None, None, None)

    # ---- in-proj: z-halves for all 3 experts on the L-quarter (epilogue gate) ----
    # silu(x) = x / (1 + exp(-x)) -- keeps ScalarE on the {exp, ln} LUT set
    silup = ctx.enter_context(tc.tile_pool(name="silu_tmp", bufs=3))

    def silu_to(dst, src_z, pn, fn):
        ex = silup.tile([pn, fn], f32, tag="silu_e", name="silu_e")
        nc.scalar.activation(ex[:], src_z, AF_.Exp, scale=-1.0)
        nc.vector.tensor_scalar_add(ex[:], ex[:], 1.0)
        nc.vector.reciprocal(ex[:], ex[:])
        nc.vector.tensor_mul(dst, src_z, ex[:])

    zq = []
    for e in range(3):
        zq_e = []
        for pi, (po, pn) in enumerate(PARTS):
            zt = big.tile([pn, QL], f32, tag=f"zq{e}{pi}", padded_shape=P128,
                          name=f"zq{e}{pi}")
            for fo, fn in FCQ:
                ps = mm_psum.tile([128, 512], f32, tag="mm", name="zq_ps")
                nc.tensor.matmul(ps[:pn, :fn], s_wz[:, e * D2 + po:e * D2 + po + pn],
                                 s_xTq[:, fo:fo + fn], start=True, stop=True)
                silu_to(zt[:, fo:fo + fn], ps[:pn, :fn], pn, fn)
            zq_e.append(zt)
        zq.append(zq_e)

    # ---- in-proj xi (local expert) into zero-padded conv buffer ----
    pads = []
    xc = []
    with tc.tile_pool(name="pad", bufs=1) as padp, \
         tc.tile_pool(name="pad_ps", bufs=1, space="PSUM") as pad_ps:
        for po, pn in PARTS:
            pad = padp.tile([pn, HS + 2, WS + 2], bf16, tag=f"pad{po}",
                            padded_shape=P128, name=f"pad{po}")
            nc.vector.memset(pad[:], 0.0)
            pads.append(pad)
        for pi, (po, pn) in enumerate(PARTS):
            ps = pad_ps.tile([128, L], f32, tag="padps", name="xi_ps")
            for fo, fn in FCH:
                nc.tensor.matmul(ps[:pn, fo:fo + fn], s_wxi[:, po:po + pn],
                                 s_xT[:, fo:fo + fn], start=True, stop=True)
            interior = pads[pi][:, 1:HS + 1, 1:WS + 1]
            nc.scalar.copy(interior, ps[:pn, :].rearrange("p (a b) -> p a b", a=HS))

        # ---- depthwise 3x3 conv + silu -> xc (flat [d, L], bf16) ----
        with tc.tile_pool(name="conv_acc", bufs=1) as accp:
            for pi, (po, pn) in enumerate(PARTS):
                xct = big.tile([pn, L], bf16, tag=f"xc{pi}", padded_shape=P128,
                               name=f"xc{pi}")
                acc = accp.tile([pn, HS, WS], bf16, tag=f"acc{pi}",
                                name=f"acc{pi}", bufs=2)
                nc.vector.tensor_scalar_mul(acc[:], pads[pi][:, 0:HS, 0:WS],
                                            s_w9[:pn, pi, 0:1])
                for tap in range(1, 9):
                    dy, dx = tap // 3, tap % 3
                    sh = pads[pi][:, dy:dy + HS, dx:dx + WS]
                    acc2 = accp.tile([pn, HS, WS], bf16, tag=f"acc{pi}",
                                     name=f"acc2{pi}", bufs=2)
                    nc.vector.scalar_tensor_tensor(
                        acc2[:], sh, s_w9[:pn, pi, tap:tap + 1], acc[:],
                        op0=OP.mult, op1=OP.add)
                    acc = acc2
                accf = acc[:].rearrange("p a b -> p (a b)")
                a2 = accp.tile([pn, L], f32, tag=f"a2{pi}", name=f"a2{pi}")
                nc.scalar.activation(a2[:], accf, AF_.Identity,
                                     bias=s_convb[:pn, pi, :])
                ex = accp.tile([pn, L], f32, tag=f"ex{pi}", name=f"ex{pi}")
                nc.scalar.activation(ex[:], accf, AF_.Exp, scale=-1.0,
                                     bias=s_convbn[:pn, pi, :])
                nc.vector.tensor_scalar_add(ex[:], ex[:], 1.0)
                nc.vector.reciprocal(ex[:], ex[:])
                nc.vector.tensor_mul(xct[:], a2[:], ex[:])
                xc.append(xct)

    # ---- transposed copy (vertical scan directions) ----
    xcT = []
    for pi, (po, pn) in enumerate(PARTS):
        t = big.tile([pn, L], bf16, tag=f"xcT{pi}", padded_shape=P128,
                     name=f"xcT{pi}")
        src = xc[pi][:].rearrange("p (h w) -> p h w", h=HS)
        srcT = bass.AP(tensor=src.tensor, offset=src.offset,
                       ap=[list(src.ap[0]), list(src.ap[2]), list(src.ap[1])])
        nc.vector.tensor_copy(t[:].rearrange("p (w h) -> p w h", w=WS), srcT)
        xcT.append(t)

    # ---- main scan loops (pools scoped: freed before the epilogue) ----
    kctx = ExitStack()
    bc_psum = kctx.enter_context(tc.tile_pool(name="bc_psum", bufs=4, space="PSUM"))
    kpool = kctx.enter_context(tc.tile_pool(name="kbufs", bufs=1))
    npool = kctx.enter_context(tc.tile_pool(name="nbufs", bufs=2))
    ypool = kctx.enter_context(tc.tile_pool(name="ypch", bufs=2))
    yacc = []
    for pi, (po, pn) in enumerate(PARTS):
        ya = big.tile([pn, L], f32, tag=f"yacc{pi}", padded_shape=P128,
                      name=f"yacc{pi}")
        nc.gpsimd.memset(ya[:], 0.0)
        yacc.append(ya)

    for k in range(4):
        rhs = xc if k in (0, 2) else xcT
        rev = k >= 2

        # x_dbl = xproj_k @ xs ; rows: [0:6]=dt, [6:22]=B, [22:38]=C
        dts = kpool.tile([R, L], bf16, tag="dts", padded_shape=P128,
                         name=f"dts{k}")
        Bm = kpool.tile([NS, L], bf16, tag="Bm", padded_shape=P128, name=f"Bm{k}")
        Cm = kpool.tile([NS, L], bf16, tag="Cm", padded_shape=P128, name=f"Cm{k}")
        for fo, fn in FCH:
            ps = mm_psum.tile([128, 512], f32, tag="mm", name=f"xd_ps{k}")
            for pi, (po, pn) in enumerate(PARTS):
                nc.tensor.matmul(ps[:CRP, :fn], s_xproj[:pn, k, pi, :],
                                 rhs[pi][:, fo:fo + fn],
                                 start=(pi == 0), stop=(pi == 1))
            nc.scalar.copy(dts[:, fo:fo + fn], ps[0:R, :fn])
            nc.scalar.copy(Bm[:, fo:fo + fn], ps[32:32 + NS, :fn])
            nc.scalar.copy(Cm[:, fo:fo + fn], ps[64:64 + NS, :fn])

        # delta = softplus(dt_w @ dts + dt_b) ; dxu = delta * xs
        delta, dxu = [], []
        for pi, (po, pn) in enumerate(PARTS):
            dl = kpool.tile([pn, L], bf16, tag=f"delta{pi}",
                            name=f"delta{pi}_{k}")
            du = kpool.tile([pn, L], bf16, tag=f"dxu{pi}",
                            name=f"dxu{pi}_{k}")
            for fo, fn in FCH:
                ps = mm_psum.tile([128, 512], f32, tag="mm", name=f"dt_ps{k}")
                nc.tensor.matmul(ps[:pn, :fn], s_dtw[:, k, po:po + pn],
                                 dts[:, fo:fo + fn], start=True, stop=True)
                # softplus(x) = ln(1 + exp(x)) on the {exp, ln} LUT set
                spe = silup.tile([pn, fn], f32, tag="spe",
                                 name=f"spe{pi}_{k}")
                nc.scalar.activation(spe[:], ps[:pn, :fn], AF_.Exp,
                                     bias=s_dtb[:pn, k, pi, :])
                nc.scalar.activation(dl[:, fo:fo + fn], spe[:], AF_.Ln, bias=1.0)
            nc.vector.tensor_mul(du[:], dl[:], rhs[pi][:])
            delta.append(dl)
            dxu.append(du)

        for n in range(NS):
            # B broadcast to 128 rows (PE), dBu = dxu * B_bc
            dBu, dA, hh = [], [], []
            for pi, (po, pn) in enumerate(PARTS):
                dBu.append(npool.tile([pn, L], bf16, tag=f"dBu{pi}",
                                      name=f"dBu{pi}_{k}_{n}"))
                dA.append(npool.tile([pn, L], bf16, tag=f"dA{pi}",
                                     name=f"dA{pi}_{k}_{n}"))
                hh.append(npool.tile([pn, L], bf16, tag=f"h{pi}",
                                     name=f"h{pi}_{k}_{n}"))
            for fo, fn in FCH:
                bb = bc_psum.tile([128, 512], f32, tag="bc", name=f"bb{k}_{n}")
                nc.tensor.matmul(bb[:, :fn], s_sel[:, n, :], Bm[:, fo:fo + fn],
                                 start=True, stop=True)
                for pi, (po, pn) in enumerate(PARTS):
                    nc.vector.tensor_mul(dBu[pi][:, fo:fo + fn],
                                         dxu[pi][:, fo:fo + fn], bb[:pn, :fn])
            for pi, (po, pn) in enumerate(PARTS):
                nc.scalar.activation(dA[pi][:], delta[pi][:], AF_.Exp,
                                     scale=s_Am[:pn, k, pi, n:n + 1])
                if rev:
                    nc.vector.tensor_tensor_scan(
                        hh[pi][:], dA[pi][:, ::-1], dBu[pi][:, ::-1], 0.0,
                        op0=OP.mult, op1=OP.add)
                else:
                    nc.vector.tensor_tensor_scan(
                        hh[pi][:], dA[pi][:], dBu[pi][:], 0.0,
                        op0=OP.mult, op1=OP.add)
            # y += h * C_bc  (h read back-to-front for reversed directions)
            for fo, fn in FCH:
                cb = bc_psum.tile([128, 512], f32, tag="bc", name=f"cb{k}_{n}")
                nc.tensor.matmul(cb[:, :fn], s_sel[:, n, :], Cm[:, fo:fo + fn],
                                 start=True, stop=True)
                for pi, (po, pn) in enumerate(PARTS):
                    yp = ypool.tile([pn, 512], f32, tag=f"yp{pi}",
                                    name=f"yp{pi}_{k}_{n}")
                    if rev:
                        hsrc = hh[pi][:, ::-1][:, fo:fo + fn]
                    else:
                        hsrc = hh[pi][:, fo:fo + fn]
                    nc.vector.tensor_mul(yp[:, :fn], hsrc, cb[:pn, :fn])
                    nc.gpsimd.tensor_add(yacc[pi][:, fo:fo + fn],
                                         yacc[pi][:, fo:fo + fn], yp[:, :fn])

    kctx.close()

    # ---- D-term (in place) + masked bf16 partials into the RS buffer ----
    for pi, (po, pn) in enumerate(PARTS):
        nc.vector.scalar_tensor_tensor(yacc[pi][:], xc[pi][:],
                                       s_dsum[:pn, pi, :], yacc[pi][:],
                                       op0=OP.mult, op1=OP.add)

    zt = consts.tile([128, QL], bf16)
    nc.vector.memset(zt[:], 0.0)
    mskp = ctx.enter_context(tc.tile_pool(name="msk", bufs=4))
    for e in range(3):
        for pi, (po, pn) in enumerate(PARTS):
            for q in range(4):
                mt = mskp.tile([pn, QL], bf16, tag="msk", name=f"msk{e}{pi}{q}")
                nc.vector.tensor_scalar_mul(mt[:], yacc[pi][:, q * QL:(q + 1) * QL],
                                            s_emask[:pn, e, :])
                nc.sync.dma_start(rs_in[q, e, pi, :pn, :], mt[:])
            if pi == 1:
                for q in range(4):
                    nc.sync.dma_start(rs_in[q, e, pi, pn:128, :],
                                      zt[:128 - pn, :])

    nc.gpsimd.collective_compute(
        "ReduceScatter", mybir.AluOpType.add,
        replica_groups=[[0, 1, 2, 3], [4, 5, 6, 7]],
        ins=[rs_in.ap().opt()], outs=[rs_out.ap().opt()])

    # ---- epilogue on this core's L-quarter (chunked to fit PSUM banks) ----
    ep = ctx.enter_context(tc.tile_pool(name="epi", bufs=2))
    ep1 = ctx.enter_context(tc.tile_pool(name="epi1", bufs=1))
    ep_ps = ctx.enter_context(tc.tile_pool(name="epi_ps", bufs=1, space="PSUM"))
    outsb = ep.tile([C, QL], f32, tag="outsb", padded_shape=P128, name="outsb")
    ye = [[None, None], [None, None], [None, None]]
    for e in range(3):
        for pi, (po, pn) in enumerate(PARTS):
            t = ep1.tile([pn, QL], f32, tag=f"ye{e}{pi}", padded_shape=P128,
                         name=f"ye{e}{pi}")
            nc.gpsimd.dma_start(t[:], rs_out[e, pi, :pn, :])
            ye[e][pi] = t
    for fo, fn in FCQ:
        outP = ep_ps.tile([C, 512], f32, tag="outP", padded_shape=P128,
                          name=f"outP{fo}")
        for e in range(3):
            # LayerNorm over d (partition axis) via PE column sums
            sums = ep_ps.tile([1, 512], f32, tag="sums", padded_shape=P128,
                              name=f"sums{e}{fo}")
            sumsq = ep_ps.tile([1, 512], f32, tag="sumsq", padded_shape=P128,
                               name=f"sumsq{e}{fo}")
            for pi, (po, pn) in enumerate(PARTS):
                s = ep.tile([pn, fn], f32, tag=f"sq{pi}", padded_shape=P128,
                            name=f"sq{e}{pi}{fo}")
                nc.vector.tensor_mul(s[:], ye[e][pi][:, fo:fo + fn],
                                     ye[e][pi][:, fo:fo + fn])
                nc.tensor.matmul(sums[:, :fn], s_onesP[:pn, :],
                                 ye[e][pi][:, fo:fo + fn],
                                 start=(pi == 0), stop=(pi == 1))
                nc.tensor.matmul(sumsq[:, :fn], s_onesP[:pn, :], s[:],
                                 start=(pi == 0), stop=(pi == 1))
            mean = ep.tile([1, fn], f32, tag="mean", padded_shape=P128,
                           name=f"mean{e}{fo}")
            nc.scalar.mul(mean[:], sums[:, :fn], 1.0 / D2)
            msq = ep.tile([1, fn], f32, tag="msq", name=f"msq{e}{fo}")
            nc.vector.tensor_mul(msq[:], mean[:], mean[:])
            varr = ep.tile([1, fn], f32, tag="varr", name=f"varr{e}{fo}")
            nc.vector.scalar_tensor_tensor(varr[:], sumsq[:, :fn], 1.0 / D2,
                                           msq[:], op0=OP.mult, op1=OP.subtract)
            # rstd = (var+eps)^-0.5 = exp(-0.5*ln(var+eps)) on {exp, ln} LUT set
            lnv = ep.tile([1, fn], f32, tag="lnv", name=f"lnv{e}{fo}")
            nc.scalar.activation(lnv[:], varr[:], AF_.Ln, bias=s_epsv[:1, :])
            rstd = ep.tile([1, fn], f32, tag="rstd", padded_shape=P128,
                           name=f"rstd{e}{fo}")
            nc.scalar.activation(rstd[:], lnv[:], AF_.Exp, scale=-0.5)
            mb_ps = ep_ps.tile([128, 512], f32, tag="mb", name=f"mb{e}{fo}")
            rb_ps = ep_ps.tile([128, 512], f32, tag="rb", name=f"rb{e}{fo}")
            nc.tensor.matmul(mb_ps[:, :fn], s_ones1[:], mean[:],
                             start=True, stop=True)
            nc.tensor.matmul(rb_ps[:, :fn], s_ones1[:], rstd[:],
                             start=True, stop=True)
            for pi, (po, pn) in enumerate(PARTS):
                cen = ep.tile([pn, fn], f32, tag=f"cen{pi}",
                              name=f"cen{e}{pi}{fo}")
                nc.vector.tensor_sub(cen[:], ye[e][pi][:, fo:fo + fn],
                                     mb_ps[:pn, :fn])
                nrm = ep.tile([pn, fn], f32, tag=f"nrm{pi}",
                              name=f"nrm{e}{pi}{fo}")
                nc.vector.tensor_mul(nrm[:], cen[:], rb_ps[:pn, :fn])
                ln = ep.tile([pn, fn], f32, tag=f"ln{pi}",
                             name=f"ln{e}{pi}{fo}")
                nc.scalar.activation(ln[:], nrm[:], AF_.Identity,
                                     scale=s_lnw[:pn, e, pi, :],
                                     bias=s_lnb[:pn, e, pi, :])
                gz = ep.tile([pn, fn], f32, tag=f"gz{pi}", padded_shape=P128,
                             name=f"gz{e}{pi}{fo}")
                nc.vector.scalar_tensor_tensor(gz[:], ln[:], gbc[:pn, e:e + 1],
                                               zq[e][pi][:, fo:fo + fn],
                                               op0=OP.mult, op1=OP.mult)
                nc.tensor.matmul(outP[:, :fn], s_outw[:pn, e, pi, :], gz[:],
                                 start=(e == 0 and pi == 0),
                                 stop=(e == 2 and pi == 1))
        nc.scalar.copy(outsb[:, fo:fo + fn], outP[:, :fn])
    nc.sync.dma_start(out_ext[:], outsb[:])
    ctx.close()


def make_core_inputs(x, g_w1, g_b1, g_w2, g_b2, ps):
    """Per-core input dicts (host-side sharding / weight slicing)."""
    import ml_dtypes
    bf = ml_dtypes.bfloat16
    ins = []
    for c in range(8):
        b, u = c // 4, c % 4
        e = UNIT_EXPERT[u]
        nlo, nhi = UNIT_NSLICE[u]
        nsz = nhi - nlo
        ne = EXPERT_N[e]
        (in_w, conv_w, conv_b, xproj_w, dt_w, dt_b, A_logs, Ds,
         ln_w, ln_b, out_w) = [np.asarray(t, np.float32) for t in ps[e]]

        xb = np.asarray(x[b], np.float32).reshape(L, C).T.copy()   # [96, L]
        xTq_ = xb[:, u * QL:(u + 1) * QL].copy()
        wxi_ = in_w[0:D2, :].T.copy()
        wz_ = np.concatenate(
            [np.asarray(ps[ee][0], np.float32)[D2:2 * D2, :].T for ee in range(3)],
            axis=1)                                                # [96, 576]
        w9_ = np.zeros((128, 2, 9), np.float32)
        cb_ = np.zeros((128, 2, 1), np.float32)
        cw = conv_w.reshape(D2, 9)
        for pi, (po, pn) in enumerate(PARTS):
            w9_[:pn, pi] = cw[po:po + pn]
            cb_[:pn, pi, 0] = conv_b[po:po + pn]
        cbn_ = -cb_
        xp_ = np.zeros((128, 4, 2, CRP), np.float32)
        dtw_ = np.zeros((R, 4, D2), np.float32)
        dtb_ = np.zeros((128, 4, 2, 1), np.float32)
        Am_ = np.zeros((128, 4, 2, NS), np.float32)
        for k in range(4):
            M = np.zeros((CRP, D2), np.float32)
            M[0:R] = xproj_w[k][0:R]
            M[32:32 + nsz] = xproj_w[k][R + nlo:R + nhi]
            M[64:64 + nsz] = xproj_w[k][R + ne + nlo:R + ne + nhi]
            A = -np.exp(A_logs[k][:, nlo:nhi])                    # [192, nsz]
            for pi, (po, pn) in enumerate(PARTS):
                xp_[:pn, k, pi] = M.T[po:po + pn]
                dtb_[:pn, k, pi, 0] = dt_b[k][po:po + pn]
                Am_[:pn, k, pi, :nsz] = A[po:po + pn]
            dtw_[:, k, :] = dt_w[k].T
        ds_ = np.zeros((128, 2, 1), np.float32)
        if u != 3:
            dall = Ds.sum(axis=0)                                 # [192]
            for pi, (po, pn) in enumerate(PARTS):
                ds_[:pn, pi, 0] = dall[po:po + pn]
        sel_ = np.zeros((NS, NS, 128), np.float32)
        for n in range(NS):
            sel_[n, n, :] = 1.0
        em_ = np.zeros((128, 3, 1), np.float32)
        em_[:, e, 0] = 1.0
        lnw_ = np.zeros((128, 3, 2, 1), np.float32)
        lnb_ = np.zeros((128, 3, 2, 1), np.float32)
        ow_ = np.zeros((128, 3, 2, C), np.float32)
        for ee in range(3):
            lw = np.asarray(ps[ee][8], np.float32)
            lb = np.asarray(ps[ee][9], np.float32)
            ow = np.asarray(ps[ee][10], np.float32)               # [96, 192]
            for pi, (po, pn) in enumerate(PARTS):
                lnw_[:pn, ee, pi, 0] = lw[po:po + pn]
                lnb_[:pn, ee, pi, 0] = lb[po:po + pn]
                ow_[:pn, ee, pi] = ow.T[po:po + pn]
        ins.append({
            "xT": xb, "xTq": xTq_, "wxi": wxi_, "wz": wz_,
            "w9": w9_, "convb": cb_, "convbn": cbn_,
            "xproj": xp_.astype(bf), "dtw": dtw_.astype(bf), "dtb": dtb_,
            "Am": Am_, "dsum": ds_, "sel16": sel_.astype(bf),
            "ones1": np.ones((1, 128), np.float32),
            "onesP": np.ones((128, 1), np.float32),
            "emask": em_, "lnw": lnw_, "lnb": lnb_, "outw": ow_,
            "gw1": np.asarray(g_w1, np.float32).T.copy(),
            "gb1": np.asarray(g_b1, np.float32).reshape(24, 1),
            "gw2": np.asarray(g_w2, np.float32).T.copy(),
            "epsv": np.full((128, 1), EPS, np.float32),
            "gb2": np.asarray(g_b2, np.float32).reshape(1, 3),
        })
    return ins


_CACHED_NC = None


LAST_EXEC_NS = None


def kernel(x, g_w1, g_b1, g_w2, g_b2, p0, p1, p2, _trace=False):
    import sys
    if '/opt/trn_rl_repo' not in sys.path:
        sys.path.insert(0, '/opt/trn_rl_repo')
    from concourse.bass_utils import run_bass_kernel_spmd

    global _CACHED_NC, LAST_EXEC_NS
    if _CACHED_NC is None:
        _CACHED_NC = build_graph()
    nc = _CACHED_NC

    in_maps = make_core_inputs(x, g_w1, g_b1, g_w2, g_b2, [p0, p1, p2])
    res = run_bass_kernel_spmd(nc, in_maps, list(range(8)), trace=_trace)
    LAST_EXEC_NS = res.exec_time_ns
    out = np.zeros((B, HS, WS, C), np.float32)
    for c in range(8):
        b, q = c // 4, c % 4
        o = np.asarray(res.results[c]["out"], np.float32)         # [96, 576]
        out[b].reshape(L, C)[q * QL:(q + 1) * QL, :] = o.T
    return out


# revision 29
# speedup vs baseline: 1.0011x; 1.0011x over previous
"""AdaptiveStateSelector (gated 3-expert SS2D / Mamba-style 2D selective scan) on 8 TRN2 cores.

Sharding: core c -> (b = c//4, unit u = c%4). Units own expert state-slices:
  u0: expert0 n=0:8 (padded to 16 lanes), u1: expert1 n=0:16, u2: expert2 n=0:16, u3: expert2 n=16:32.
Each core computes all 4 scan directions for its slice; direction flips/transposes are
handled in-core (negative-stride APs for flipped scans, a transposed copy for the
vertical scans). Partial gated outputs are combined with one ReduceScatter per
batch-group (experts as slots, masked by per-core ownership data), after which every
core runs the epilogue (LayerNorm, silu gate, gated out-projection) on its L-quarter.
"""

import numpy as np

B, HS, WS, C = 2, 48, 48, 96
L = HS * WS            # 2304
D2 = 192               # expanded channels (2*96)
NS = 16                # n-lanes per core (padded)
R = 6                  # dt rank
CR = R + 2 * NS        # 38 live rows of the sliced x-proj
CRP = 96               # padded to 32-aligned sections: dt@0, B@32, C@64
QL = L // 4            # 576, L-quarter per core in the epilogue
PARTS = [(0, 128), (128, 64)]     # d=192 split into partition tiles
FCH = [(0, 512), (512, 512), (1024, 512), (1536, 512), (2048, 256)]
FCQ = [(0, 512), (512, 64)]       # 576 in PSUM-bank chunks
UNIT_EXPERT = [0, 1, 2, 2]
UNIT_NSLICE = [(0, 8), (0, 16), (0, 16), (16, 32)]
EXPERT_N = [8, 16, 32]
EPS = 1e-5


def build_graph():
    import concourse.bass as bass
    import concourse.tile as tile
    from concourse import bacc, mybir

    f32 = mybir.dt.float32
    bf16 = mybir.dt.bfloat16
    AF = mybir.ActivationFunctionType
    OP = mybir.AluOpType

    nc = bacc.Bacc("TRN2", target_bir_lowering=False, debug=False, num_devices=8)

    def param(name, shape, dt=f32, out=False):
        return nc.declare_dram_parameter(name, list(shape), dt, isOutput=out)

    params = dict(
        xT=param("xT", [C, L]),
        xTq=param("xTq", [C, QL]),
        wxi=param("wxi", [C, D2]),
        wz=param("wz", [C, 3 * D2]),
        w9=param("w9", [128, 2, 9]),
        convb=param("convb", [128, 2, 1]),
        convbn=param("convbn", [128, 2, 1]),
        xproj=param("xproj", [128, 4, 2, CRP], bf16),
        dtw=param("dtw", [R, 4, D2], bf16),
        dtb=param("dtb", [128, 4, 2, 1]),
        Am=param("Am", [128, 4, 2, NS]),
        dsum=param("dsum", [128, 2, 1]),
        sel16=param("sel16", [NS, NS, 128], bf16),
        ones1=param("ones1", [1, 128]),
        onesP=param("onesP", [128, 1]),
        emask=param("emask", [128, 3, 1]),
        lnw=param("lnw", [128, 3, 2, 1]),
        lnb=param("lnb", [128, 3, 2, 1]),
        outw=param("outw", [128, 3, 2, C]),
        gw1=param("gw1", [C, 24]),
        gb1=param("gb1", [24, 1]),
        gw2=param("gw2", [24, 3]),
        gb2=param("gb2", [1, 3]),
        epsv=param("epsv", [128, 1]),
        out_ext=param("out", [C, QL], out=True),
    )
    params["rs_in"] = nc.dram_tensor("rs_in", [4, 3, 2, 128, QL], bf16)
    params["rs_out"] = nc.dram_tensor("rs_out", [3, 2, 128, QL], bf16)

    with tile.TileContext(nc) as tc:
        _build(nc, tc, bass, mybir, tile, f32, bf16, AF, OP, params)
    nc.compile()
    return nc


def _build(nc, tc, bass, mybir, tile, f32, bf16, AF, OP, T):
    from contextlib import ExitStack

    ctx = ExitStack()
    AF_ = AF
    P128 = [128, Ellipsis]

    consts = ctx.enter_context(tc.tile_pool(name="consts", bufs=1))
    big = ctx.enter_context(tc.tile_pool(name="big", bufs=1))
    mm_psum = ctx.enter_context(tc.tile_pool(name="mm_psum", bufs=3, space="PSUM"))

    # ---- load constants to SBUF (all padded to 128 partitions) ----
    def load_const(pname, dt=f32):
        p = T[pname]
        t = consts.tile(list(p.shape), dt, padded_shape=P128, name=f"s_{pname}")
        nc.sync.dma_start(t[:], p[:])
        return t

    s_xT = load_const("xT")
    s_xTq = load_const("xTq")
    s_wxi = load_const("wxi")
    s_wz = load_const("wz")
    s_w9 = load_const("w9")
    s_convb = load_const("convb")
    s_convbn = load_const("convbn")
    s_xproj = load_const("xproj", bf16)
    s_dtw = load_const("dtw", bf16)
    s_dtb = load_const("dtb")
    s_Am = load_const("Am")
    s_dsum = load_const("dsum")
    s_sel = load_const("sel16", bf16)
    s_ones1 = load_const("ones1")
    s_onesP = load_const("onesP")
    s_emask = load_const("emask")
    s_lnw = load_const("lnw")
    s_lnb = load_const("lnb")
    s_outw = load_const("outw")
    s_gw1 = load_const("gw1")
    s_gb1 = load_const("gb1")
    s_gw2 = load_const("gw2")
    s_gb2 = load_const("gb2")
    s_epsv = load_const("epsv")
    out_ext, rs_in, rs_out = T["out_ext"], T["rs_in"], T["rs_out"]

    # ---- gating network (redundant on every core) ----
    gp = ctx.enter_context(tc.tile_pool(name="gates", bufs=1))
    gates_ps_ctx = tc.tile_pool(name="gates_ps", bufs=1, space="PSUM")
    gp_ps = gates_ps_ctx.__enter__()
    pooled = gp.tile([C, 1], f32, padded_shape=P128)
    nc.vector.tensor_reduce(pooled[:], s_xT[:], axis=mybir.AxisListType.X, op=OP.add)
    h1p = gp_ps.tile([24, 1], f32, padded_shape=P128)
    nc.tensor.matmul(h1p[:], s_gw1[:], pooled[:], start=True, stop=True)
    h1 = gp.tile([24, 1], f32, padded_shape=P128)
    nc.scalar.activation(h1[:], h1p[:], AF_.Relu, bias=s_gb1[:], scale=1.0 / L)
    logp = gp_ps.tile([1, 3], f32, padded_shape=P128)
    nc.tensor.matmul(logp[:], h1[:], s_gw2[:], start=True, stop=True)
    logits = gp.tile([1, 3], f32, padded_shape=P128)
    nc.vector.tensor_add(logits[:], logp[:], s_gb2[:])
    lmax = gp.tile([1, 1], f32, padded_shape=P128)
    nc.vector.tensor_reduce(lmax[:], logits[:], axis=mybir.AxisListType.X, op=OP.max,
                            negate=True)
    elog = gp.tile([1, 3], f32, padded_shape=P128)
    nc.scalar.activation(elog[:], logits[:], AF_.Exp, bias=lmax[:])
    esum = gp.tile([1, 1], f32, padded_shape=P128)
    nc.vector.tensor_reduce(esum[:], elog[:], axis=mybir.AxisListType.X, op=OP.add)
    einv = gp.tile([1, 1], f32, padded_shape=P128)
    nc.vector.reciprocal(einv[:], esum[:])
    gates = gp.tile([1, 3], f32, padded_shape=P128)
    nc.vector.tensor_scalar_mul(gates[:], elog[:], einv[:])
    gbc_ps = gp_ps.tile([128, 3], f32)
    nc.tensor.matmul(gbc_ps[:], s_ones1[:], gates[:], start=True, stop=True)
    gbc = gp.tile([128, 3], f32)
    nc.scalar.copy(gbc[:], gbc_ps[:])
    gates_ps_ctx.__exit__(None, None, None)

    # ---- in-proj: z-halves for all 3 experts on the L-quarter (epilogue gate) ----
    # silu(x) = x / (1 + exp(-x)) -- keeps ScalarE on the {exp, ln} LUT set
    silup = ctx.enter_context(tc.tile_pool(name="silu_tmp", bufs=3))

    def silu_to(dst, src_z, pn, fn):
        ex = silup.tile([pn, fn], f32, tag="silu_e", name="silu_e")
        nc.scalar.activation(ex[:], src_z, AF_.Exp, scale=-1.0)
        nc.vector.tensor_scalar_add(ex[:], ex[:], 1.0)
        nc.vector.reciprocal(ex[:], ex[:])
        nc.vector.tensor_mul(dst, src_z, ex[:])

    zq = []
    for e in range(3):
        zq_e = []
        for pi, (po, pn) in enumerate(PARTS):
            zt = big.tile([pn, QL], f32, tag=f"zq{e}{pi}", padded_shape=P128,
                          name=f"zq{e}{pi}")
            for fo, fn in FCQ:
                ps = mm_psum.tile([128, 512], f32, tag="mm", name="zq_ps")
                nc.tensor.matmul(ps[:pn, :fn], s_wz[:, e * D2 + po:e * D2 + po + pn],
                                 s_xTq[:, fo:fo + fn], start=True, stop=True)
                silu_to(zt[:, fo:fo + fn], ps[:pn, :fn], pn, fn)
            zq_e.append(zt)
        zq.append(zq_e)

    # ---- in-proj xi (local expert) into zero-padded conv buffer ----
    pads = []
    xc = []
    with tc.tile_pool(name="pad", bufs=1) as padp, \
         tc.tile_pool(name="pad_ps", bufs=1, space="PSUM") as pad_ps:
        for po, pn in PARTS:
            pad = padp.tile([pn, HS + 2, WS + 2], bf16, tag=f"pad{po}",
                            padded_shape=P128, name=f"pad{po}")
            nc.vector.memset(pad[:], 0.0)
            pads.append(pad)
        for pi, (po, pn) in enumerate(PARTS):
            ps = pad_ps.tile([128, L], f32, tag="padps", name="xi_ps")
            for fo, fn in FCH:
                nc.tensor.matmul(ps[:pn, fo:fo + fn], s_wxi[:, po:po + pn],
                                 s_xT[:, fo:fo + fn], start=True, stop=True)
            interior = pads[pi][:, 1:HS + 1, 1:WS + 1]
            nc.scalar.copy(interior, ps[:pn, :].rearrange("p (a b) -> p a b", a=HS))

        # ---- depthwise 3x3 conv + silu -> xc (flat [d, L], bf16) ----
        with tc.tile_pool(name="conv_acc", bufs=1) as accp:
            for pi, (po, pn) in enumerate(PARTS):
                xct = big.tile([pn, L], bf16, tag=f"xc{pi}", padded_shape=P128,
                               name=f"xc{pi}")
                acc = accp.tile([pn, HS, WS], bf16, tag=f"acc{pi}",
                                name=f"acc{pi}", bufs=2)
                nc.vector.tensor_scalar_mul(acc[:], pads[pi][:, 0:HS, 0:WS],
                                            s_w9[:pn, pi, 0:1])
                for tap in range(1, 9):
                    dy, dx = tap // 3, tap % 3
                    sh = pads[pi][:, dy:dy + HS, dx:dx + WS]
                    acc2 = accp.tile([pn, HS, WS], bf16, tag=f"acc{pi}",
                                     name=f"acc2{pi}", bufs=2)
                    nc.vector.scalar_tensor_tensor(
                        acc2[:], sh, s_w9[:pn, pi, tap:tap + 1], acc[:],
                        op0=OP.mult, op1=OP.add)
                    acc = acc2
                accf = acc[:].rearrange("p a b -> p (a b)")
                a2 = accp.tile([pn, L], f32, tag=f"a2{pi}", name=f"a2{pi}")
                nc.scalar.activation(a2[:], accf, AF_.Identity,
                                     bias=s_convb[:pn, pi, :])
                ex = accp.tile([pn, L], f32, tag=f"ex{pi}", name=f"ex{pi}")
                nc.scalar.activation(ex[:], accf, AF_.Exp, scale=-1.0,
                                     bias=s_convbn[:pn, pi, :])
                nc.vector.tensor_scalar_add(ex[:], ex[:], 1.0)
                nc.vector.reciprocal(ex[:], ex[:])
                nc.vector.tensor_mul(xct[:], a2[:], ex[:])
                xc.append(xct)

    # ---- transposed copy (vertical scan directions) ----
    xcT = []
    for pi, (po, pn) in enumerate(PARTS):
        t = big.tile([pn, L], bf16, tag=f"xcT{pi}", padded_shape=P128,
                     name=f"xcT{pi}")
        src = xc[pi][:].rearrange("p (h w) -> p h w", h=HS)
        srcT = bass.AP(tensor=src.tensor, offset=src.offset,
                       ap=[list(src.ap[0]), list(src.ap[2]), list(src.ap[1])])
        nc.vector.tensor_copy(t[:].rearrange("p (w h) -> p w h", w=WS), srcT)
        xcT.append(t)

    # ---- main scan loops (pools scoped: freed before the epilogue) ----
    kctx = ExitStack()
    bc_psum = kctx.enter_context(tc.tile_pool(name="bc_psum", bufs=4, space="PSUM"))
    kpool = kctx.enter_context(tc.tile_pool(name="kbufs", bufs=1))
    npool = kctx.enter_context(tc.tile_pool(name="nbufs", bufs=2))
    ypool = kctx.enter_context(tc.tile_pool(name="ypch", bufs=2))
    class Bal:
        # deterministic DVE/GpSimd load balancer (same on every core)
        def __init__(self):
            self.dve = 0.0
            self.gps = 0.0

        def pick(self, dve_cost, gps_factor=2.0):
            # scans are VectorE-only (Pool codegen rejects TensorTensorScan)
            self.dve += dve_cost
            return nc.vector

        def pick_add(self, fn, gps_factor=1.85):
            c = (fn / 512.0) * 0.62
            if self.dve <= self.gps:
                self.dve += c
                return nc.vector
            self.gps += c * gps_factor
            return nc.gpsimd

    bal = Bal()
    yacc = []
    for pi, (po, pn) in enumerate(PARTS):
        ya = big.tile([pn, L], f32, tag=f"yacc{pi}", padded_shape=P128,
                      name=f"yacc{pi}")
        nc.gpsimd.memset(ya[:], 0.0)
        yacc.append(ya)

    for k in range(4):
        rhs = xc if k in (0, 2) else xcT
        rev = k >= 2

        # x_dbl = xproj_k @ xs ; rows: [0:6]=dt, [6:22]=B, [22:38]=C
        dts = kpool.tile([R, L], bf16, tag="dts", padded_shape=P128,
                         name=f"dts{k}")
        Bm = kpool.tile([NS, L], bf16, tag="Bm", padded_shape=P128, name=f"Bm{k}")
        Cm = kpool.tile([NS, L], bf16, tag="Cm", padded_shape=P128, name=f"Cm{k}")
        for fo, fn in FCH:
            ps = mm_psum.tile([128, 512], f32, tag="mm", name=f"xd_ps{k}")
            for pi, (po, pn) in enumerate(PARTS):
                nc.tensor.matmul(ps[:CRP, :fn], s_xproj[:pn, k, pi, :],
                                 rhs[pi][:, fo:fo + fn],
                                 start=(pi == 0), stop=(pi == 1))
            nc.scalar.copy(dts[:, fo:fo + fn], ps[0:R, :fn])
            nc.scalar.copy(Bm[:, fo:fo + fn], ps[32:32 + NS, :fn])
            nc.scalar.copy(Cm[:, fo:fo + fn], ps[64:64 + NS, :fn])

        # delta = softplus(dt_w @ dts + dt_b) ; dxu = delta * xs
        delta, dxu = [], []
        for pi, (po, pn) in enumerate(PARTS):
            dl = kpool.tile([pn, L], bf16, tag=f"delta{pi}",
                            name=f"delta{pi}_{k}")
            du = kpool.tile([pn, L], bf16, tag=f"dxu{pi}",
                            name=f"dxu{pi}_{k}")
            for fo, fn in FCH:
                ps = mm_psum.tile([128, 512], f32, tag="mm", name=f"dt_ps{k}")
                nc.tensor.matmul(ps[:pn, :fn], s_dtw[:, k, po:po + pn],
                                 dts[:, fo:fo + fn], start=True, stop=True)
                # softplus(x) = ln(1 + exp(x)) on the {exp, ln} LUT set
                spe = silup.tile([pn, fn], f32, tag="spe",
                                 name=f"spe{pi}_{k}")
                nc.scalar.activation(spe[:], ps[:pn, :fn], AF_.Exp,
                                     bias=s_dtb[:pn, k, pi, :])
                nc.scalar.activation(dl[:, fo:fo + fn], spe[:], AF_.Ln, bias=1.0)
            nc.vector.tensor_mul(du[:], dl[:], rhs[pi][:])
            delta.append(dl)
            dxu.append(du)

        pair = {}
        for n in range(NS):
            half = n % 2
            # part0 full tile [128, L]; part1 packed as halves of a pair tile
            dBu0 = npool.tile([128, L], bf16, tag="dBu0", name=f"dBu0_{k}_{n}")
            dA0 = npool.tile([128, L], bf16, tag="dA0", name=f"dA0_{k}_{n}")
            h0 = npool.tile([128, L], bf16, tag="h0", name=f"h0_{k}_{n}")
            if half == 0:
                pair = dict(
                    dBu=npool.tile([128, L], bf16, tag="dBu1", name=f"dBu1_{k}_{n}"),
                    dA=npool.tile([128, L], bf16, tag="dA1", name=f"dA1_{k}_{n}"),
                    h=npool.tile([128, L], bf16, tag="h1", name=f"h1_{k}_{n}"),
                    Csav=npool.tile([64, L], bf16, tag="Csav", name=f"Csav_{k}_{n}"),
                )
            rows = slice(half * 64, half * 64 + 64)
            for fo, fn in FCH:
                bb = bc_psum.tile([128, 512], f32, tag="bc", name=f"bb{k}_{n}")
                nc.tensor.matmul(bb[:, :fn], s_sel[:, n, :], Bm[:, fo:fo + fn],
                                 start=True, stop=True)
                nc.vector.tensor_mul(dBu0[:, fo:fo + fn],
                                     dxu[0][:, fo:fo + fn], bb[:, :fn])
                nc.vector.tensor_mul(pair["dBu"][rows, fo:fo + fn],
                                     dxu[1][:, fo:fo + fn], bb[:64, :fn])
            nc.scalar.activation(dA0[:], delta[0][:], AF_.Exp,
                                 scale=s_Am[:, k, 0, n:n + 1])
            nc.scalar.activation(pair["dA"][rows, :], delta[1][:], AF_.Exp,
                                 scale=s_Am[:64, k, 1, n:n + 1])
            scan_eng = bal.pick(4.93)
            if rev:
                scan_eng.tensor_tensor_scan(
                    h0[:], dA0[:, ::-1], dBu0[:, ::-1], 0.0,
                    op0=OP.mult, op1=OP.add)
            else:
                scan_eng.tensor_tensor_scan(
                    h0[:], dA0[:], dBu0[:], 0.0, op0=OP.mult, op1=OP.add)
            if half == 1:
                scan_eng = bal.pick(4.93)
                if rev:
                    scan_eng.tensor_tensor_scan(
                        pair["h"][:], pair["dA"][:, ::-1], pair["dBu"][:, ::-1],
                        0.0, op0=OP.mult, op1=OP.add)
                else:
                    scan_eng.tensor_tensor_scan(
                        pair["h"][:], pair["dA"][:], pair["dBu"][:], 0.0,
                        op0=OP.mult, op1=OP.add)
            # y accumulation: yacc += h * C_bc
            for fo, fn in FCH:
                cb = bc_psum.tile([128, 512], f32, tag="bc", name=f"cb{k}_{n}")
                nc.tensor.matmul(cb[:, :fn], s_sel[:, n, :], Cm[:, fo:fo + fn],
                                 start=True, stop=True)
                if half == 0:
                    # save C rows for the deferred part1-half0 product
                    nc.scalar.copy(pair["Csav"][:, fo:fo + fn], cb[:64, :fn])
                yp = ypool.tile([128, 512], f32, tag="yp", name=f"yp_{k}_{n}")
                hsrc = h0[:, ::-1][:, fo:fo + fn] if rev else h0[:, fo:fo + fn]
                nc.vector.tensor_mul(yp[:, :fn], hsrc, cb[:, :fn])
                bal.pick_add(fn).tensor_add(yacc[0][:, fo:fo + fn],
                                            yacc[0][:, fo:fo + fn], yp[:, :fn])
                if half == 1:
                    hp = pair["h"]
                    yp1 = ypool.tile([64, 512], f32, tag="yp1",
                                     name=f"yp1a_{k}_{n}")
                    hs = (hp[:, ::-1] if rev else hp[:])[0:64, fo:fo + fn]
                    nc.vector.tensor_mul(yp1[:, :fn], hs,
                                         pair["Csav"][:, fo:fo + fn])
                    bal.pick_add(fn).tensor_add(yacc[1][:, fo:fo + fn],
                                                yacc[1][:, fo:fo + fn],
                                                yp1[:, :fn])
                    yp2 = ypool.tile([64, 512], f32, tag="yp2",
                                     name=f"yp2_{k}_{n}")
                    hs = (hp[:, ::-1] if rev else hp[:])[64:128, fo:fo + fn]
                    nc.vector.tensor_mul(yp2[:, :fn], hs, cb[:64, :fn])
                    bal.pick_add(fn).tensor_add(yacc[1][:, fo:fo + fn],
                                                yacc[1][:, fo:fo + fn],
                                                yp2[:, :fn])

    kctx.close()

    # ---- D-term (in place) + masked bf16 partials into the RS buffer ----
    for pi, (po, pn) in enumerate(PARTS):
        nc.vector.scalar_tensor_tensor(yacc[pi][:], xc[pi][:],
                                       s_dsum[:pn, pi, :], yacc[pi][:],
                                       op0=OP.mult, op1=OP.add)

    zt = consts.tile([128, QL], bf16)
    nc.vector.memset(zt[:], 0.0)
    mskp = ctx.enter_context(tc.tile_pool(name="msk", bufs=4))
    for e in range(3):
        for pi, (po, pn) in enumerate(PARTS):
            for q in range(4):
                mt = mskp.tile([pn, QL], bf16, tag="msk", name=f"msk{e}{pi}{q}")
                nc.vector.tensor_scalar_mul(mt[:], yacc[pi][:, q * QL:(q + 1) * QL],
                                            s_emask[:pn, e, :])
                nc.sync.dma_start(rs_in[q, e, pi, :pn, :], mt[:])
            if pi == 1:
                for q in range(4):
                    nc.sync.dma_start(rs_in[q, e, pi, pn:128, :],
                                      zt[:128 - pn, :])

    nc.gpsimd.collective_compute(
        "ReduceScatter", mybir.AluOpType.add,
        replica_groups=[[0, 1, 2, 3], [4, 5, 6, 7]],
        ins=[rs_in.ap().opt()], outs=[rs_out.ap().opt()])

    # ---- epilogue on this core's L-quarter (chunked to fit PSUM banks) ----
    ep = ctx.enter_context(tc.tile_pool(name="epi", bufs=2))
    ep1 = ctx.enter_context(tc.tile_pool(name="epi1", bufs=1))
    ep_ps = ctx.enter_context(tc.tile_pool(name="epi_ps", bufs=1, space="PSUM"))
    outsb = ep.tile([C, QL], f32, tag="outsb", padded_shape=P128, name="outsb")
    ye = [[None, None], [None, None], [None, None]]
    for e in range(3):
        for pi, (po, pn) in enumerate(PARTS):
            t = ep1.tile([pn, QL], f32, tag=f"ye{e}{pi}", padded_shape=P128,
                         name=f"ye{e}{pi}")
            nc.gpsimd.dma_start(t[:], rs_out[e, pi, :pn, :])
            ye[e][pi] = t
    for fo, fn in FCQ:
        outP = ep_ps.tile([C, 512], f32, tag="outP", padded_shape=P128,
                          name=f"outP{fo}")
        for e in range(3):
            # LayerNorm over d (partition axis) via PE column sums
            sums = ep_ps.tile([1, 512], f32, tag="sums", padded_shape=P128,
                              name=f"sums{e}{fo}")
            sumsq = ep_ps.tile([1, 512], f32, tag="sumsq", padded_shape=P128,
                               name=f"sumsq{e}{fo}")
            for pi, (po, pn) in enumerate(PARTS):
                s = ep.tile([pn, fn], f32, tag=f"sq{pi}", padded_shape=P128,
                            name=f"sq{e}{pi}{fo}")
                nc.vector.tensor_mul(s[:], ye[e][pi][:, fo:fo + fn],
                                     ye[e][pi][:, fo:fo + fn])
                nc.tensor.matmul(sums[:, :fn], s_onesP[:pn, :],
                                 ye[e][pi][:, fo:fo + fn],
                                 start=(pi == 0), stop=(pi == 1))
                nc.tensor.matmul(sumsq[:, :fn], s_onesP[:pn, :], s[:],
                                 start=(pi == 0), stop=(pi == 1))
            mean = ep.tile([1, fn], f32, tag="mean", padded_shape=P128,
                           name=f"mean{e}{fo}")
            nc.scalar.mul(mean[:], sums[:, :fn], 1.0 / D2)
            msq = ep.tile([1, fn], f32, tag="msq", name=f"msq{e}{fo}")
            nc.vector.tensor_mul(msq[:], mean[:], mean[:])
            varr = ep.tile([1, fn], f32, tag="varr", name=f"varr{e}{fo}")
            nc.vector.scalar_tensor_tensor(varr[:], sumsq[:, :fn], 1.0 / D2,
                                           msq[:], op0=OP.mult, op1=OP.subtract)
            # rstd = (var+eps)^-0.5 = exp(-0.5*ln(var+eps)) on {exp, ln} LUT set
            lnv = ep.tile([1, fn], f32, tag="lnv", name=f"lnv{e}{fo}")
            nc.scalar.activation(lnv[:], varr[:], AF_.Ln, bias=s_epsv[:1, :])
            rstd = ep.tile([1, fn], f32, tag="rstd", padded_shape=P128,
                           name=f"rstd{e}{fo}")
            nc.scalar.activation(rstd[:], lnv[:], AF_.Exp, scale=-0.5)
            mb_ps = ep_ps.tile([128, 512], f32, tag="mb", name=f"mb{e}{fo}")
            rb_ps = ep_ps.tile([128, 512], f32, tag="rb", name=f"rb{e}{fo}")
            nc.tensor.matmul(mb_ps[:, :fn], s_ones1[:], mean[:],
                             start=True, stop=True)
            nc.tensor.matmul(rb_ps[:, :fn], s_ones1[:], rstd[:],
                             start=True, stop=True)
            for pi, (po, pn) in enumerate(PARTS):
                cen = ep.tile([pn, fn], f32, tag=f"cen{pi}",
                              name=f"cen{e}{pi}{fo}")
                nc.vector.tensor_sub(cen[:], ye[e][pi][:, fo:fo + fn],
                                     mb_ps[:pn, :fn])
                nrm = ep.tile([pn, fn], f32, tag=f"nrm{pi}",
                              name=f"nrm{e}{pi}{fo}")
                nc.vector.tensor_mul(nrm[:], cen[:], rb_ps[:pn, :fn])
                ln = ep.tile([pn, fn], f32, tag=f"ln{pi}",
                             name=f"ln{e}{pi}{fo}")
                nc.scalar.activation(ln[:], nrm[:], AF_.Identity,
                                     scale=s_lnw[:pn, e, pi, :],
                                     bias=s_lnb[:pn, e, pi, :])
                gz = ep.tile([pn, fn], f32, tag=f"gz{pi}", padded_shape=P128,
                             name=f"gz{e}{pi}{fo}")
                nc.vector.scalar_tensor_tensor(gz[:], ln[:], gbc[:pn, e:e + 1],
                                               zq[e][pi][:, fo:fo + fn],
                                               op0=OP.mult, op1=OP.mult)
                nc.tensor.matmul(outP[:, :fn], s_outw[:pn, e, pi, :], gz[:],
                                 start=(e == 0 and pi == 0),
                                 stop=(e == 2 and pi == 1))
        nc.scalar.copy(outsb[:, fo:fo + fn], outP[:, :fn])
    nc.sync.dma_start(out_ext[:], outsb[:])
    ctx.close()


def make_core_inputs(x, g_w1, g_b1, g_w2, g_b2, ps):
    """Per-core input dicts (host-side sharding / weight slicing)."""
    import ml_dtypes
    bf = ml_dtypes.bfloat16
    ins = []
    for c in range(8):
        b, u = c // 4, c % 4
        e = UNIT_EXPERT[u]
        nlo, nhi = UNIT_NSLICE[u]
        nsz = nhi - nlo
        ne = EXPERT_N[e]
        (in_w, conv_w, conv_b, xproj_w, dt_w, dt_b, A_logs, Ds,
         ln_w, ln_b, out_w) = [np.asarray(t, np.float32) for t in ps[e]]

        xb = np.asarray(x[b], np.float32).reshape(L, C).T.copy()   # [96, L]
        xTq_ = xb[:, u * QL:(u + 1) * QL].copy()
        wxi_ = in_w[0:D2, :].T.copy()
        wz_ = np.concatenate(
            [np.asarray(ps[ee][0], np.float32)[D2:2 * D2, :].T for ee in range(3)],
            axis=1)                                                # [96, 576]
        w9_ = np.zeros((128, 2, 9), np.float32)
        cb_ = np.zeros((128, 2, 1), np.float32)
        cw = conv_w.reshape(D2, 9)
        for pi, (po, pn) in enumerate(PARTS):
            w9_[:pn, pi] = cw[po:po + pn]
            cb_[:pn, pi, 0] = conv_b[po:po + pn]
        cbn_ = -cb_
        xp_ = np.zeros((128, 4, 2, CRP), np.float32)
        dtw_ = np.zeros((R, 4, D2), np.float32)
        dtb_ = np.zeros((128, 4, 2, 1), np.float32)
        Am_ = np.zeros((128, 4, 2, NS), np.float32)
        for k in range(4):
            M = np.zeros((CRP, D2), np.float32)
            M[0:R] = xproj_w[k][0:R]
            M[32:32 + nsz] = xproj_w[k][R + nlo:R + nhi]
            M[64:64 + nsz] = xproj_w[k][R + ne + nlo:R + ne + nhi]
            A = -np.exp(A_logs[k][:, nlo:nhi])                    # [192, nsz]
            for pi, (po, pn) in enumerate(PARTS):
                xp_[:pn, k, pi] = M.T[po:po + pn]
                dtb_[:pn, k, pi, 0] = dt_b[k][po:po + pn]
                Am_[:pn, k, pi, :nsz] = A[po:po + pn]
            dtw_[:, k, :] = dt_w[k].T
        ds_ = np.zeros((128, 2, 1), np.float32)
        if u != 3:
            dall = Ds.sum(axis=0)                                 # [192]
            for pi, (po, pn) in enumerate(PARTS):
                ds_[:pn, pi, 0] = dall[po:po + pn]
        sel_ = np.zeros((NS, NS, 128), np.float32)
        for n in range(NS):
            sel_[n, n, :] = 1.0
        em_ = np.zeros((128, 3, 1), np.float32)
        em_[:, e, 0] = 1.0
        lnw_ = np.zeros((128, 3, 2, 1), np.float32)
        lnb_ = np.zeros((128, 3, 2, 1), np.float32)
        ow_ = np.zeros((128, 3, 2, C), np.float32)
        for ee in range(3):
            lw = np.asarray(ps[ee][8], np.float32)
            lb = np.asarray(ps[ee][9], np.float32)
            ow = np.asarray(ps[ee][10], np.float32)               # [96, 192]
            for pi, (po, pn) in enumerate(PARTS):
                lnw_[:pn, ee, pi, 0] = lw[po:po + pn]
                lnb_[:pn, ee, pi, 0] = lb[po:po + pn]
                ow_[:pn, ee, pi] = ow.T[po:po + pn]
        ins.append({
            "xT": xb, "xTq": xTq_, "wxi": wxi_, "wz": wz_,
            "w9": w9_, "convb": cb_, "convbn": cbn_,
            "xproj": xp_.astype(bf), "dtw": dtw_.astype(bf), "dtb": dtb_,
            "Am": Am_, "dsum": ds_, "sel16": sel_.astype(bf),
            "ones1": np.ones((1, 128), np.float32),
            "onesP": np.ones((128, 1), np.float32),
            "emask": em_, "lnw": lnw_, "lnb": lnb_, "outw": ow_,
            "gw1": np.asarray(g_w1, np.float32).T.copy(),
            "gb1": np.asarray(g_b1, np.float32).reshape(24, 1),
            "gw2": np.asarray(g_w2, np.float32).T.copy(),
            "epsv": np.full((128, 1), EPS, np.float32),
            "gb2": np.asarray(g_b2, np.float32).reshape(1, 3),
        })
    return ins


_CACHED_NC = None


LAST_EXEC_NS = None


def kernel(x, g_w1, g_b1, g_w2, g_b2, p0, p1, p2, _trace=False):
    import sys
    if '/opt/trn_rl_repo' not in sys.path:
        sys.path.insert(0, '/opt/trn_rl_repo')
    from concourse.bass_utils import run_bass_kernel_spmd

    global _CACHED_NC, LAST_EXEC_NS
    if _CACHED_NC is None:
        _CACHED_NC = build_graph()
    nc = _CACHED_NC

    in_maps = make_core_inputs(x, g_w1, g_b1, g_w2, g_b2, [p0, p1, p2])
    res = run_bass_kernel_spmd(nc, in_maps, list(range(8)), trace=_trace)
    LAST_EXEC_NS = res.exec_time_ns
    out = np.zeros((B, HS, WS, C), np.float32)
    for c in range(8):
        b, q = c // 4, c % 4
        o = np.asarray(res.results[c]["out"], np.float32)         # [96, 576]
        out[b].reshape(L, C)[q * QL:(q + 1) * QL, :] = o.T
    return out


# revision 33
# speedup vs baseline: 1.1054x; 1.1042x over previous
"""AdaptiveStateSelector (gated 3-expert SS2D / Mamba-style 2D selective scan) on 8 TRN2 cores.

Sharding: core c -> (b = c//4, unit u = c%4). Units own expert state-slices:
  u0: expert0 n=0:8 (padded to 16 lanes), u1: expert1 n=0:16, u2: expert2 n=0:16, u3: expert2 n=16:32.
Each core computes all 4 scan directions for its slice; direction flips/transposes are
handled in-core (negative-stride APs for flipped scans, a transposed copy for the
vertical scans). Partial gated outputs are combined with one ReduceScatter per
batch-group (experts as slots, masked by per-core ownership data), after which every
core runs the epilogue (LayerNorm, silu gate, gated out-projection) on its L-quarter.
"""

import numpy as np

B, HS, WS, C = 2, 48, 48, 96
L = HS * WS            # 2304
D2 = 192               # expanded channels (2*96)
NS = 16                # n-lanes per core (padded)
R = 6                  # dt rank
CR = R + 2 * NS        # 38 live rows of the sliced x-proj
CRP = 96               # padded to 32-aligned sections: dt@0, B@32, C@64
QL = L // 4            # 576, L-quarter per core in the epilogue
PARTS = [(0, 128), (128, 64)]     # d=192 split into partition tiles
FCH = [(0, 512), (512, 512), (1024, 512), (1536, 512), (2048, 256)]
FCQ = [(0, 512), (512, 64)]       # 576 in PSUM-bank chunks
UNIT_EXPERT = [0, 1, 2, 2]
UNIT_NSLICE = [(0, 8), (0, 16), (0, 16), (16, 32)]
EXPERT_N = [8, 16, 32]
EPS = 1e-5


def build_graph():
    import concourse.bass as bass
    import concourse.tile as tile
    from concourse import bacc, mybir

    f32 = mybir.dt.float32
    bf16 = mybir.dt.bfloat16
    AF = mybir.ActivationFunctionType
    OP = mybir.AluOpType

    nc = bacc.Bacc("TRN2", target_bir_lowering=False, debug=False, num_devices=8)

    def param(name, shape, dt=f32, out=False):
        return nc.declare_dram_parameter(name, list(shape), dt, isOutput=out)

    params = dict(
        xT=param("xT", [C, L]),
        xTq=param("xTq", [C, QL]),
        wxi=param("wxi", [C, D2]),
        wz=param("wz", [C, 3 * D2]),
        w9=param("w9", [128, 2, 9]),
        convb=param("convb", [128, 2, 1]),
        convbn=param("convbn", [128, 2, 1]),
        xproj=param("xproj", [128, 4, 2, CRP], bf16),
        dtw=param("dtw", [R, 4, D2], bf16),
        dtb=param("dtb", [128, 4, 2, 1]),
        Am=param("Am", [128, 4, 2, NS]),
        dsum=param("dsum", [128, 2, 1]),
        sel16=param("sel16", [NS, NS, 128], bf16),
        ones1=param("ones1", [1, 128]),
        onesP=param("onesP", [128, 1]),
        emask=param("emask", [128, 3, 1]),
        lnw=param("lnw", [128, 3, 2, 1]),
        lnb=param("lnb", [128, 3, 2, 1]),
        outw=param("outw", [128, 3, 2, C]),
        gw1=param("gw1", [C, 24]),
        gb1=param("gb1", [24, 1]),
        gw2=param("gw2", [24, 3]),
        gb2=param("gb2", [1, 3]),
        epsv=param("epsv", [128, 1]),
        out_ext=param("out", [C, QL], out=True),
    )
    params["rs_in"] = nc.dram_tensor("rs_in", [4, 3, 2, 128, QL], bf16)
    params["rs_out"] = nc.dram_tensor("rs_out", [3, 2, 128, QL], bf16)

    with tile.TileContext(nc) as tc:
        _build(nc, tc, bass, mybir, tile, f32, bf16, AF, OP, params)
    nc.compile()
    return nc


def _build(nc, tc, bass, mybir, tile, f32, bf16, AF, OP, T):
    from contextlib import ExitStack

    ctx = ExitStack()
    AF_ = AF
    P128 = [128, Ellipsis]

    consts = ctx.enter_context(tc.tile_pool(name="consts", bufs=1))
    big = ctx.enter_context(tc.tile_pool(name="big", bufs=1))
    mm_psum = ctx.enter_context(tc.tile_pool(name="mm_psum", bufs=3, space="PSUM"))

    # ---- load constants to SBUF (all padded to 128 partitions) ----
    def load_const(pname, dt=f32):
        p = T[pname]
        t = consts.tile(list(p.shape), dt, padded_shape=P128, name=f"s_{pname}")
        nc.sync.dma_start(t[:], p[:])
        return t

    s_xT = load_const("xT")
    s_xTq = load_const("xTq")
    s_wxi = load_const("wxi")
    s_wz = load_const("wz")
    s_w9 = load_const("w9")
    s_convb = load_const("convb")
    s_convbn = load_const("convbn")
    s_xproj = load_const("xproj", bf16)
    s_dtw = load_const("dtw", bf16)
    s_dtb = load_const("dtb")
    s_Am = load_const("Am")
    s_dsum = load_const("dsum")
    s_sel = load_const("sel16", bf16)
    s_ones1 = load_const("ones1")
    s_onesP = load_const("onesP")
    s_emask = load_const("emask")
    s_lnw = load_const("lnw")
    s_lnb = load_const("lnb")
    s_outw = load_const("outw")
    s_gw1 = load_const("gw1")
    s_gb1 = load_const("gb1")
    s_gw2 = load_const("gw2")
    s_gb2 = load_const("gb2")
    s_epsv = load_const("epsv")
    out_ext, rs_in, rs_out = T["out_ext"], T["rs_in"], T["rs_out"]

    # ---- gating network (redundant on every core) ----
    gp = ctx.enter_context(tc.tile_pool(name="gates", bufs=1))
    gates_ps_ctx = tc.tile_pool(name="gates_ps", bufs=1, space="PSUM")
    gp_ps = gates_ps_ctx.__enter__()
    pooled = gp.tile([C, 1], f32, padded_shape=P128)
    nc.vector.tensor_reduce(pooled[:], s_xT[:], axis=mybir.AxisListType.X, op=OP.add)
    h1p = gp_ps.tile([24, 1], f32, padded_shape=P128)
    nc.tensor.matmul(h1p[:], s_gw1[:], pooled[:], start=True, stop=True)
    h1 = gp.tile([24, 1], f32, padded_shape=P128)
    nc.scalar.activation(h1[:], h1p[:], AF_.Relu, bias=s_gb1[:], scale=1.0 / L)
    logp = gp_ps.tile([1, 3], f32, padded_shape=P128)
    nc.tensor.matmul(logp[:], h1[:], s_gw2[:], start=True, stop=True)
    logits = gp.tile([1, 3], f32, padded_shape=P128)
    nc.vector.tensor_add(logits[:], logp[:], s_gb2[:])
    lmax = gp.tile([1, 1], f32, padded_shape=P128)
    nc.vector.tensor_reduce(lmax[:], logits[:], axis=mybir.AxisListType.X, op=OP.max,
                            negate=True)
    elog = gp.tile([1, 3], f32, padded_shape=P128)
    nc.scalar.activation(elog[:], logits[:], AF_.Exp, bias=lmax[:])
    esum = gp.tile([1, 1], f32, padded_shape=P128)
    nc.vector.tensor_reduce(esum[:], elog[:], axis=mybir.AxisListType.X, op=OP.add)
    einv = gp.tile([1, 1], f32, padded_shape=P128)
    nc.vector.reciprocal(einv[:], esum[:])
    gates = gp.tile([1, 3], f32, padded_shape=P128)
    nc.vector.tensor_scalar_mul(gates[:], elog[:], einv[:])
    gbc_ps = gp_ps.tile([128, 3], f32)
    nc.tensor.matmul(gbc_ps[:], s_ones1[:], gates[:], start=True, stop=True)
    gbc = gp.tile([128, 3], f32)
    nc.scalar.copy(gbc[:], gbc_ps[:])
    gates_ps_ctx.__exit__(None, None, None)

    # ---- in-proj: z-halves for all 3 experts on the L-quarter (epilogue gate) ----
    # silu(x) = x / (1 + exp(-x)) -- keeps ScalarE on the {exp, ln} LUT set
    silup = ctx.enter_context(tc.tile_pool(name="silu_tmp", bufs=3))

    def silu_to(dst, src_z, pn, fn):
        ex = silup.tile([pn, fn], f32, tag="silu_e", name="silu_e")
        nc.scalar.activation(ex[:], src_z, AF_.Exp, scale=-1.0)
        nc.vector.tensor_scalar_add(ex[:], ex[:], 1.0)
        nc.vector.reciprocal(ex[:], ex[:])
        nc.vector.tensor_mul(dst, src_z, ex[:])

    zq = []
    for e in range(3):
        zq_e = []
        for pi, (po, pn) in enumerate(PARTS):
            zt = big.tile([pn, QL], f32, tag=f"zq{e}{pi}", padded_shape=P128,
                          name=f"zq{e}{pi}")
            for fo, fn in FCQ:
                ps = mm_psum.tile([128, 512], f32, tag="mm", name="zq_ps")
                nc.tensor.matmul(ps[:pn, :fn], s_wz[:, e * D2 + po:e * D2 + po + pn],
                                 s_xTq[:, fo:fo + fn], start=True, stop=True)
                silu_to(zt[:, fo:fo + fn], ps[:pn, :fn], pn, fn)
            zq_e.append(zt)
        zq.append(zq_e)

    # ---- in-proj xi (local expert) into zero-padded conv buffer ----
    pads = []
    xc = []
    with tc.tile_pool(name="pad", bufs=1) as padp, \
         tc.tile_pool(name="pad_ps", bufs=1, space="PSUM") as pad_ps:
        for po, pn in PARTS:
            pad = padp.tile([pn, HS + 2, WS + 2], bf16, tag=f"pad{po}",
                            padded_shape=P128, name=f"pad{po}")
            nc.vector.memset(pad[:], 0.0)
            pads.append(pad)
        for pi, (po, pn) in enumerate(PARTS):
            ps = pad_ps.tile([128, L], f32, tag="padps", name="xi_ps")
            for fo, fn in FCH:
                nc.tensor.matmul(ps[:pn, fo:fo + fn], s_wxi[:, po:po + pn],
                                 s_xT[:, fo:fo + fn], start=True, stop=True)
            interior = pads[pi][:, 1:HS + 1, 1:WS + 1]
            nc.scalar.copy(interior, ps[:pn, :].rearrange("p (a b) -> p a b", a=HS))

        # ---- depthwise 3x3 conv + silu -> xc (flat [d, L], bf16) ----
        with tc.tile_pool(name="conv_acc", bufs=1) as accp:
            for pi, (po, pn) in enumerate(PARTS):
                xct = big.tile([pn, L], bf16, tag=f"xc{pi}", padded_shape=P128,
                               name=f"xc{pi}")
                acc = accp.tile([pn, HS, WS], bf16, tag=f"acc{pi}",
                                name=f"acc{pi}", bufs=2)
                nc.vector.tensor_scalar_mul(acc[:], pads[pi][:, 0:HS, 0:WS],
                                            s_w9[:pn, pi, 0:1])
                for tap in range(1, 9):
                    dy, dx = tap // 3, tap % 3
                    sh = pads[pi][:, dy:dy + HS, dx:dx + WS]
                    acc2 = accp.tile([pn, HS, WS], bf16, tag=f"acc{pi}",
                                     name=f"acc2{pi}", bufs=2)
                    nc.vector.scalar_tensor_tensor(
                        acc2[:], sh, s_w9[:pn, pi, tap:tap + 1], acc[:],
                        op0=OP.mult, op1=OP.add)
                    acc = acc2
                accf = acc[:].rearrange("p a b -> p (a b)")
                a2 = accp.tile([pn, L], f32, tag=f"a2{pi}", name=f"a2{pi}")
                nc.scalar.activation(a2[:], accf, AF_.Identity,
                                     bias=s_convb[:pn, pi, :])
                ex = accp.tile([pn, L], f32, tag=f"ex{pi}", name=f"ex{pi}")
                nc.scalar.activation(ex[:], accf, AF_.Exp, scale=-1.0,
                                     bias=s_convbn[:pn, pi, :])
                nc.vector.tensor_scalar_add(ex[:], ex[:], 1.0)
                nc.vector.reciprocal(ex[:], ex[:])
                nc.vector.tensor_mul(xct[:], a2[:], ex[:])
                xc.append(xct)

    # ---- transposed copy (vertical scan directions) ----
    xcT = []
    for pi, (po, pn) in enumerate(PARTS):
        t = big.tile([pn, L], bf16, tag=f"xcT{pi}", padded_shape=P128,
                     name=f"xcT{pi}")
        src = xc[pi][:].rearrange("p (h w) -> p h w", h=HS)
        srcT = bass.AP(tensor=src.tensor, offset=src.offset,
                       ap=[list(src.ap[0]), list(src.ap[2]), list(src.ap[1])])
        nc.vector.tensor_copy(t[:].rearrange("p (w h) -> p w h", w=WS), srcT)
        xcT.append(t)

    # ---- main scan loops (pools scoped: freed before the epilogue) ----
    kctx = ExitStack()
    bc_psum = kctx.enter_context(tc.tile_pool(name="bc_psum", bufs=4, space="PSUM"))
    kpool = kctx.enter_context(tc.tile_pool(name="kbufs", bufs=1))
    npool = kctx.enter_context(tc.tile_pool(name="nbufs", bufs=2))
    ypool = kctx.enter_context(tc.tile_pool(name="ypch", bufs=2))
    class Bal:
        # deterministic DVE/GpSimd load balancer (same on every core)
        def __init__(self):
            self.dve = 0.0
            self.gps = 0.0

        def _choose(self, dve_cost, gps_cost):
            if self.dve + dve_cost <= self.gps + gps_cost:
                self.dve += dve_cost
                return nc.vector
            self.gps += gps_cost
            return nc.gpsimd

        def pick_tt(self, fn):
            c = fn / 2304.0
            return self._choose(1.26 * c, 3.1 * c)

        def pick_add(self, fn, pn=128):
            c = fn / 512.0
            return self._choose(0.62 * c, 1.37 * c)

    bal = Bal()
    yacc = []
    for pi, (po, pn) in enumerate(PARTS):
        ya = big.tile([pn, L], f32, tag=f"yacc{pi}", padded_shape=P128,
                      name=f"yacc{pi}")
        nc.gpsimd.memset(ya[:], 0.0)
        yacc.append(ya)

    for k in range(4):
        rhs = xc if k in (0, 2) else xcT
        rev = k >= 2

        # x_dbl = xproj_k @ xs ; rows: [0:6]=dt, [6:22]=B, [22:38]=C
        dts = kpool.tile([R, L], bf16, tag="dts", padded_shape=P128,
                         name=f"dts{k}")
        Bm = kpool.tile([NS, L], bf16, tag="Bm", padded_shape=P128, name=f"Bm{k}")
        Cm = kpool.tile([NS, L], bf16, tag="Cm", padded_shape=P128, name=f"Cm{k}")
        for fo, fn in FCH:
            ps = mm_psum.tile([128, 512], f32, tag="mm", name=f"xd_ps{k}")
            for pi, (po, pn) in enumerate(PARTS):
                nc.tensor.matmul(ps[:CRP, :fn], s_xproj[:pn, k, pi, :],
                                 rhs[pi][:, fo:fo + fn],
                                 start=(pi == 0), stop=(pi == 1))
            nc.scalar.copy(dts[:, fo:fo + fn], ps[0:R, :fn])
            nc.scalar.copy(Bm[:, fo:fo + fn], ps[32:32 + NS, :fn])
            nc.scalar.copy(Cm[:, fo:fo + fn], ps[64:64 + NS, :fn])

        # delta = softplus(dt_w @ dts + dt_b) ; dxu = delta * xs
        delta, dxu = [], []
        for pi, (po, pn) in enumerate(PARTS):
            dl = kpool.tile([pn, L], bf16, tag=f"delta{pi}",
                            name=f"delta{pi}_{k}")
            du = kpool.tile([pn, L], bf16, tag=f"dxu{pi}",
                            name=f"dxu{pi}_{k}")
            for fo, fn in FCH:
                ps = mm_psum.tile([128, 512], f32, tag="mm", name=f"dt_ps{k}")
                nc.tensor.matmul(ps[:pn, :fn], s_dtw[:, k, po:po + pn],
                                 dts[:, fo:fo + fn], start=True, stop=True)
                # softplus(x) = ln(1 + exp(x)) on the {exp, ln} LUT set
                spe = silup.tile([pn, fn], f32, tag="spe",
                                 name=f"spe{pi}_{k}")
                nc.scalar.activation(spe[:], ps[:pn, :fn], AF_.Exp,
                                     bias=s_dtb[:pn, k, pi, :])
                nc.scalar.activation(dl[:, fo:fo + fn], spe[:], AF_.Ln, bias=1.0)
            nc.vector.tensor_mul(du[:], dl[:], rhs[pi][:])
            delta.append(dl)
            dxu.append(du)

        pair = {}
        for n in range(NS):
            half = n % 2
            dBu0 = npool.tile([128, L], bf16, tag="dBu0", name=f"dBu0_{k}_{n}")
            dA0 = npool.tile([128, L], bf16, tag="dA0", name=f"dA0_{k}_{n}")
            h0 = npool.tile([128, L], bf16, tag="h0", name=f"h0_{k}_{n}")
            Bbs = npool.tile([128, L], bf16, tag="Bbs", name=f"Bbs_{k}_{n}")
            Cbs = npool.tile([128, L], bf16, tag="Cbs", name=f"Cbs_{k}_{n}")
            if half == 0:
                pair = dict(
                    dBu=npool.tile([128, L], bf16, tag="dBu1", bufs=1,
                                   name=f"dBu1_{k}_{n}"),
                    dA=npool.tile([128, L], bf16, tag="dA1", bufs=1,
                                  name=f"dA1_{k}_{n}"),
                    h=npool.tile([128, L], bf16, tag="h1", name=f"h1_{k}_{n}"),
                )
            rows = slice(half * 64, half * 64 + 64)
            # B/C broadcasts -> PSUM -> SBUF bf16 (copies on ScalarE)
            for fo, fn in FCH:
                bb = bc_psum.tile([128, 512], f32, tag="bc", name=f"bb{k}_{n}")
                nc.tensor.matmul(bb[:, :fn], s_sel[:, n, :], Bm[:, fo:fo + fn],
                                 start=True, stop=True)
                nc.scalar.copy(Bbs[:, fo:fo + fn], bb[:, :fn])
                cb = bc_psum.tile([128, 512], f32, tag="bc", name=f"cb{k}_{n}")
                nc.tensor.matmul(cb[:, :fn], s_sel[:, n, :], Cm[:, fo:fo + fn],
                                 start=True, stop=True)
                nc.scalar.copy(Cbs[:, fo:fo + fn], cb[:, :fn])
            # dBu = dxu * B (full-tile bf16 2x ops, DVE/GPS balanced)
            bal.pick_tt(2304).tensor_mul(dBu0[:], dxu[0][:], Bbs[:])
            bal.pick_tt(2304).tensor_mul(pair["dBu"][rows, :], dxu[1][:],
                                         Bbs[:64, :])
            nc.scalar.activation(dA0[:], delta[0][:], AF_.Exp,
                                 scale=s_Am[:, k, 0, n:n + 1])
            nc.scalar.activation(pair["dA"][rows, :], delta[1][:], AF_.Exp,
                                 scale=s_Am[:64, k, 1, n:n + 1])
            if rev:
                nc.vector.tensor_tensor_scan(
                    h0[:], dA0[:, ::-1], dBu0[:, ::-1], 0.0,
                    op0=OP.mult, op1=OP.add)
            else:
                nc.vector.tensor_tensor_scan(
                    h0[:], dA0[:], dBu0[:], 0.0, op0=OP.mult, op1=OP.add)
            if half == 1:
                if rev:
                    nc.vector.tensor_tensor_scan(
                        pair["h"][:], pair["dA"][:, ::-1], pair["dBu"][:, ::-1],
                        0.0, op0=OP.mult, op1=OP.add)
                else:
                    nc.vector.tensor_tensor_scan(
                        pair["h"][:], pair["dA"][:], pair["dBu"][:], 0.0,
                        op0=OP.mult, op1=OP.add)
            # yP = h * C (full-tile bf16 2x), then chunked adds into yacc
            yp0 = ypool.tile([128, L], bf16, tag="yp0", name=f"yp0_{k}_{n}")
            h0src = h0[:, ::-1] if rev else h0[:]
            bal.pick_tt(2304).tensor_mul(yp0[:], h0src, Cbs[:])
            for fo, fn in FCH:
                bal.pick_add(fn).tensor_add(yacc[0][:, fo:fo + fn],
                                            yacc[0][:, fo:fo + fn],
                                            yp0[:, fo:fo + fn])
            if half == 1:
                hp = pair["h"][:, ::-1] if rev else pair["h"][:]
                yp1 = ypool.tile([64, L], bf16, tag="yp12", name=f"yp1_{k}_{n}")
                yp2 = ypool.tile([64, L], bf16, tag="yp12", name=f"yp2_{k}_{n}")
                bal.pick_tt(2304).tensor_mul(yp1[:], hp[0:64, :], Cprev[:64, :])
                bal.pick_tt(2304).tensor_mul(yp2[:], hp[64:128, :], Cbs[64:128, :])
                for fo, fn in FCH:
                    bal.pick_add(fn, 64).tensor_add(
                        yacc[1][:, fo:fo + fn], yacc[1][:, fo:fo + fn],
                        yp1[:, fo:fo + fn])
                    bal.pick_add(fn, 64).tensor_add(
                        yacc[1][:, fo:fo + fn], yacc[1][:, fo:fo + fn],
                        yp2[:, fo:fo + fn])
            Cprev = Cbs

    kctx.close()

    # ---- D-term (in place) + masked bf16 partials into the RS buffer ----
    for pi, (po, pn) in enumerate(PARTS):
        nc.vector.scalar_tensor_tensor(yacc[pi][:], xc[pi][:],
                                       s_dsum[:pn, pi, :], yacc[pi][:],
                                       op0=OP.mult, op1=OP.add)

    zt = consts.tile([128, QL], bf16)
    nc.vector.memset(zt[:], 0.0)
    mskp = ctx.enter_context(tc.tile_pool(name="msk", bufs=4))
    for e in range(3):
        for pi, (po, pn) in enumerate(PARTS):
            for q in range(4):
                mt = mskp.tile([pn, QL], bf16, tag="msk", name=f"msk{e}{pi}{q}")
                nc.vector.tensor_scalar_mul(mt[:], yacc[pi][:, q * QL:(q + 1) * QL],
                                            s_emask[:pn, e, :])
                nc.sync.dma_start(rs_in[q, e, pi, :pn, :], mt[:])
            if pi == 1:
                for q in range(4):
                    nc.sync.dma_start(rs_in[q, e, pi, pn:128, :],
                                      zt[:128 - pn, :])

    nc.gpsimd.collective_compute(
        "ReduceScatter", mybir.AluOpType.add,
        replica_groups=[[0, 1, 2, 3], [4, 5, 6, 7]],
        ins=[rs_in.ap().opt()], outs=[rs_out.ap().opt()])

    # ---- epilogue on this core's L-quarter (chunked to fit PSUM banks) ----
    ep = ctx.enter_context(tc.tile_pool(name="epi", bufs=2))
    ep1 = ctx.enter_context(tc.tile_pool(name="epi1", bufs=1))
    ep_ps = ctx.enter_context(tc.tile_pool(name="epi_ps", bufs=1, space="PSUM"))
    outsb = ep.tile([C, QL], f32, tag="outsb", padded_shape=P128, name="outsb")
    ye = [[None, None], [None, None], [None, None]]
    for e in range(3):
        for pi, (po, pn) in enumerate(PARTS):
            t = ep1.tile([pn, QL], f32, tag=f"ye{e}{pi}", padded_shape=P128,
                         name=f"ye{e}{pi}")
            nc.gpsimd.dma_start(t[:], rs_out[e, pi, :pn, :])
            ye[e][pi] = t
    for fo, fn in FCQ:
        outP = ep_ps.tile([C, 512], f32, tag="outP", padded_shape=P128,
                          name=f"outP{fo}")
        for e in range(3):
            # LayerNorm over d (partition axis) via PE column sums
            sums = ep_ps.tile([1, 512], f32, tag="sums", padded_shape=P128,
                              name=f"sums{e}{fo}")
            sumsq = ep_ps.tile([1, 512], f32, tag="sumsq", padded_shape=P128,
                               name=f"sumsq{e}{fo}")
            for pi, (po, pn) in enumerate(PARTS):
                s = ep.tile([pn, fn], f32, tag=f"sq{pi}", padded_shape=P128,
                            name=f"sq{e}{pi}{fo}")
                nc.vector.tensor_mul(s[:], ye[e][pi][:, fo:fo + fn],
                                     ye[e][pi][:, fo:fo + fn])
                nc.tensor.matmul(sums[:, :fn], s_onesP[:pn, :],
                                 ye[e][pi][:, fo:fo + fn],
                                 start=(pi == 0), stop=(pi == 1))
                nc.tensor.matmul(sumsq[:, :fn], s_onesP[:pn, :], s[:],
                                 start=(pi == 0), stop=(pi == 1))
            mean = ep.tile([1, fn], f32, tag="mean", padded_shape=P128,
                           name=f"mean{e}{fo}")
            nc.scalar.mul(mean[:], sums[:, :fn], 1.0 / D2)
            msq = ep.tile([1, fn], f32, tag="msq", name=f"msq{e}{fo}")
            nc.vector.tensor_mul(msq[:], mean[:], mean[:])
            varr = ep.tile([1, fn], f32, tag="varr", name=f"varr{e}{fo}")
            nc.vector.scalar_tensor_tensor(varr[:], sumsq[:, :fn], 1.0 / D2,
                                           msq[:], op0=OP.mult, op1=OP.subtract)
            # rstd = (var+eps)^-0.5 = exp(-0.5*ln(var+eps)) on {exp, ln} LUT set
            lnv = ep.tile([1, fn], f32, tag="lnv", name=f"lnv{e}{fo}")
            nc.scalar.activation(lnv[:], varr[:], AF_.Ln, bias=s_epsv[:1, :])
            rstd = ep.tile([1, fn], f32, tag="rstd", padded_shape=P128,
                           name=f"rstd{e}{fo}")
            nc.scalar.activation(rstd[:], lnv[:], AF_.Exp, scale=-0.5)
            mb_ps = ep_ps.tile([128, 512], f32, tag="mb", name=f"mb{e}{fo}")
            rb_ps = ep_ps.tile([128, 512], f32, tag="rb", name=f"rb{e}{fo}")
            nc.tensor.matmul(mb_ps[:, :fn], s_ones1[:], mean[:],
                             start=True, stop=True)
            nc.tensor.matmul(rb_ps[:, :fn], s_ones1[:], rstd[:],
                             start=True, stop=True)
            for pi, (po, pn) in enumerate(PARTS):
                cen = ep.tile([pn, fn], f32, tag=f"cen{pi}",
                              name=f"cen{e}{pi}{fo}")
                nc.vector.tensor_sub(cen[:], ye[e][pi][:, fo:fo + fn],
                                     mb_ps[:pn, :fn])
                nrm = ep.tile([pn, fn], f32, tag=f"nrm{pi}",
                              name=f"nrm{e}{pi}{fo}")
                nc.vector.tensor_mul(nrm[:], cen[:], rb_ps[:pn, :fn])
                ln = ep.tile([pn, fn], f32, tag=f"ln{pi}",
                             name=f"ln{e}{pi}{fo}")
                nc.scalar.activation(ln[:], nrm[:], AF_.Identity,
                                     scale=s_lnw[:pn, e, pi, :],
                                     bias=s_lnb[:pn, e, pi, :])
                gz = ep.tile([pn, fn], f32, tag=f"gz{pi}", padded_shape=P128,
                             name=f"gz{e}{pi}{fo}")
                nc.vector.scalar_tensor_tensor(gz[:], ln[:], gbc[:pn, e:e + 1],
                                               zq[e][pi][:, fo:fo + fn],
                                               op0=OP.mult, op1=OP.mult)
                nc.tensor.matmul(outP[:, :fn], s_outw[:pn, e, pi, :], gz[:],
                                 start=(e == 0 and pi == 0),
                                 stop=(e == 2 and pi == 1))
        nc.scalar.copy(outsb[:, fo:fo + fn], outP[:, :fn])
    nc.sync.dma_start(out_ext[:], outsb[:])
    ctx.close()


def make_core_inputs(x, g_w1, g_b1, g_w2, g_b2, ps):
    """Per-core input dicts (host-side sharding / weight slicing)."""
    import ml_dtypes
    bf = ml_dtypes.bfloat16
    ins = []
    for c in range(8):
        b, u = c // 4, c % 4
        e = UNIT_EXPERT[u]
        nlo, nhi = UNIT_NSLICE[u]
        nsz = nhi - nlo
        ne = EXPERT_N[e]
        (in_w, conv_w, conv_b, xproj_w, dt_w, dt_b, A_logs, Ds,
         ln_w, ln_b, out_w) = [np.asarray(t, np.float32) for t in ps[e]]

        xb = np.asarray(x[b], np.float32).reshape(L, C).T.copy()   # [96, L]
        xTq_ = xb[:, u * QL:(u + 1) * QL].copy()
        wxi_ = in_w[0:D2, :].T.copy()
        wz_ = np.concatenate(
            [np.asarray(ps[ee][0], np.float32)[D2:2 * D2, :].T for ee in range(3)],
            axis=1)                                                # [96, 576]
        w9_ = np.zeros((128, 2, 9), np.float32)
        cb_ = np.zeros((128, 2, 1), np.float32)
        cw = conv_w.reshape(D2, 9)
        for pi, (po, pn) in enumerate(PARTS):
            w9_[:pn, pi] = cw[po:po + pn]
            cb_[:pn, pi, 0] = conv_b[po:po + pn]
        cbn_ = -cb_
        xp_ = np.zeros((128, 4, 2, CRP), np.float32)
        dtw_ = np.zeros((R, 4, D2), np.float32)
        dtb_ = np.zeros((128, 4, 2, 1), np.float32)
        Am_ = np.zeros((128, 4, 2, NS), np.float32)
        for k in range(4):
            M = np.zeros((CRP, D2), np.float32)
            M[0:R] = xproj_w[k][0:R]
            M[32:32 + nsz] = xproj_w[k][R + nlo:R + nhi]
            M[64:64 + nsz] = xproj_w[k][R + ne + nlo:R + ne + nhi]
            A = -np.exp(A_logs[k][:, nlo:nhi])                    # [192, nsz]
            for pi, (po, pn) in enumerate(PARTS):
                xp_[:pn, k, pi] = M.T[po:po + pn]
                dtb_[:pn, k, pi, 0] = dt_b[k][po:po + pn]
                Am_[:pn, k, pi, :nsz] = A[po:po + pn]
            dtw_[:, k, :] = dt_w[k].T
        ds_ = np.zeros((128, 2, 1), np.float32)
        if u != 3:
            dall = Ds.sum(axis=0)                                 # [192]
            for pi, (po, pn) in enumerate(PARTS):
                ds_[:pn, pi, 0] = dall[po:po + pn]
        sel_ = np.zeros((NS, NS, 128), np.float32)
        for n in range(NS):
            sel_[n, n, :] = 1.0
        em_ = np.zeros((128, 3, 1), np.float32)
        em_[:, e, 0] = 1.0
        lnw_ = np.zeros((128, 3, 2, 1), np.float32)
        lnb_ = np.zeros((128, 3, 2, 1), np.float32)
        ow_ = np.zeros((128, 3, 2, C), np.float32)
        for ee in range(3):
            lw = np.asarray(ps[ee][8], np.float32)
            lb = np.asarray(ps[ee][9], np.float32)
            ow = np.asarray(ps[ee][10], np.float32)               # [96, 192]
            for pi, (po, pn) in enumerate(PARTS):
                lnw_[:pn, ee, pi, 0] = lw[po:po + pn]
                lnb_[:pn, ee, pi, 0] = lb[po:po + pn]
                ow_[:pn, ee, pi] = ow.T[po:po + pn]
        ins.append({
            "xT": xb, "xTq": xTq_, "wxi": wxi_, "wz": wz_,
            "w9": w9_, "convb": cb_, "convbn": cbn_,
            "xproj": xp_.astype(bf), "dtw": dtw_.astype(bf), "dtb": dtb_,
            "Am": Am_, "dsum": ds_, "sel16": sel_.astype(bf),
            "ones1": np.ones((1, 128), np.float32),
            "onesP": np.ones((128, 1), np.float32),
            "emask": em_, "lnw": lnw_, "lnb": lnb_, "outw": ow_,
            "gw1": np.asarray(g_w1, np.float32).T.copy(),
            "gb1": np.asarray(g_b1, np.float32).reshape(24, 1),
            "gw2": np.asarray(g_w2, np.float32).T.copy(),
            "epsv": np.full((128, 1), EPS, np.float32),
            "gb2": np.asarray(g_b2, np.float32).reshape(1, 3),
        })
    return ins


_CACHED_NC = None


LAST_EXEC_NS = None


def kernel(x, g_w1, g_b1, g_w2, g_b2, p0, p1, p2, _trace=False):
    import sys
    if '/opt/trn_rl_repo' not in sys.path:
        sys.path.insert(0, '/opt/trn_rl_repo')
    from concourse.bass_utils import run_bass_kernel_spmd

    global _CACHED_NC, LAST_EXEC_NS
    if _CACHED_NC is None:
        _CACHED_NC = build_graph()
    nc = _CACHED_NC

    in_maps = make_core_inputs(x, g_w1, g_b1, g_w2, g_b2, [p0, p1, p2])
    res = run_bass_kernel_spmd(nc, in_maps, list(range(8)), trace=_trace)
    LAST_EXEC_NS = res.exec_time_ns
    out = np.zeros((B, HS, WS, C), np.float32)
    for c in range(8):
        b, q = c // 4, c % 4
        o = np.asarray(res.results[c]["out"], np.float32)         # [96, 576]
        out[b].reshape(L, C)[q * QL:(q + 1) * QL, :] = o.T
    return out
